# revision 30
# baseline (speedup 1.0000x reference)
"""MoE QKV parallel linear for Trainium2, 8 NeuronCores.

Problem: out[t] = x[t] @ W[id[t]].T with x [16384, 2048] f32,
W [4, 3072, 2048] f32, id sorted int32 (tokens pre-grouped by expert).

Sharding: data-parallel over tokens, fixed equal shards. Core c gets
tokens [c*2048, (c+1)*2048) and computes a dense [2048, 2048] @
[2048, 3072] matmul against its majority expert's weight (no padding, 16
m-tiles exactly); the few boundary tokens whose expert differs (~100 for
near-uniform counts) are recomputed exactly on the host and patched in.
Falls back to padded expert-pure shards under heavy skew. Host transposes
x-shards and W and casts both to bf16 (absmax-rel error of the
bf16 x bf16 / f32-accum matmul is ~2.2e-3, vs the 2e-2 gate).

Device kernel (per core), all sized to hardware-measured costs:
- bf16 operands: matmuls lower to LDWEIGHTS+MATMUL pairs the PE reorder
  window pipelines (f32r matmuls stay self-loading and pay the weight
  load serially, ~25% slower).
- x^T fully SBUF-resident as [128, 512] tiles; W^T streamed once in three
  1024-wide column superblocks, next superblock prefetched 2 tiles/block.
- m-blocks of 2 token-tiles: 4 PSUM banks live per block, the other 4
  spare, so PSUM recycling never stalls the accumulation (k-outer,
  start=k0/stop=k15, f32 accumulation).
- All PSUM->SBUF copies on DVE (its queue has nothing else; HWDGE
  dma_start triggers cost ~0.6us of the issuing engine and head-block it),
  two 512-chunks staged into one [128, 1024] tile per store.
- Store triggers lag two blocks so early completion-semaphore lanes only
  pace loads; startup x/W tiles interleave across both HWDGE rings; 12
  zero matmuls pre-warm the HAM clock gate during the initial DMA wait.
"""

import numpy as np

import concourse.bacc as bacc
import concourse.mybir as mybir
import concourse.tile as tile

NCORES = 8
HIDDEN = 2048
QKV_OUT = 3072
P = 128
KO = HIDDEN // P          # 16 contraction tiles
NCHUNK = 512              # PSUM free dim per matmul (fp32 max)
SBW = 1024                # W column superblock width
NSB = QKV_OUT // SBW      # 3 superblocks
MBLK = 2                  # m-tiles per block (4 PSUM banks live, 4 spare)
XTW = 4 * P               # x resident tile width (tokens)

_cache: dict = {}


def _blocks(mt: int):
    """Partition mt m-tiles into blocks of MBLK, ending with two 1-tile
    blocks so the final PSUM drain (copies + last store) is short."""
    out = []
    m0 = 0
    while mt - m0 > 2:
        out.append((m0, MBLK))
        m0 += MBLK
    while m0 < mt:
        out.append((m0, 1))
        m0 += 1
    return out


def _build(mt: int):
    """Bass module for one core: out[mt*128, 3072] = xT.T @ wT (bf16 in)."""
    nc = bacc.Bacc("TRN2", target_bir_lowering=False, debug=False)
    tmax = mt * P
    bf16 = mybir.dt.bfloat16
    f32 = mybir.dt.float32

    # Host pre-arranges x^T and W^T as [partition, k-tile, col] so a single
    # >=1MB DMA (the line-rate threshold) can deliver many k-tiles at once.
    xT = nc.dram_tensor("xT", [P, KO, tmax], bf16, kind="ExternalInput")
    wT = nc.dram_tensor("wT", [P, KO, QKV_OUT], bf16, kind="ExternalInput")
    out = nc.dram_tensor("out", [tmax, QKV_OUT], f32, kind="ExternalOutput")

    blocks = _blocks(mt)
    nxf, xrem = divmod(tmax, XTW)  # full x q-spans + ragged tail

    with tile.TileContext(nc) as tc:
        with (
            tc.tile_pool(name="xa", bufs=max(1, nxf - 1)) as xa,
            tc.tile_pool(name="xq0", bufs=KO // 4) as xq0p,
            tc.tile_pool(name="xr", bufs=1) as xr,
            tc.tile_pool(name="wp", bufs=2 * (KO // 4)) as wp,
            tc.tile_pool(name="pp", bufs=8, space="PSUM") as pp,
            tc.tile_pool(name="op", bufs=8) as op,
        ):
            # Resident x^T: tiles[k][q] of [128, 512] bf16 on the sync ring,
            # q-major so arrival matches block consumption order.
            # PE warmup: the HAM clock gate starts at K=4/8 (1.2 GHz) and
            # needs ~3.4us of sustained matmul activity to release. The PE
            # is idle for the first ~10us waiting on DMA anyway, so run
            # dummy matmuls on zeroed scratch to enter the kernel warm.
            wl = wp.tile([P, P], bf16, name="warm_l", tag="warm_l", bufs=1)
            wr = wp.tile([P, NCHUNK], bf16, name="warm_r", tag="warm_r",
                         bufs=1)
            nc.vector.memset(wl[:], 0.0)
            nc.vector.memset(wr[:], 0.0)
            psw = pp.tile([P, NCHUNK], f32, name="warm_ps", tag="ps")
            for _ in range(12):
                nc.tensor.matmul(psw[:], wl[:], wr[:], start=True, stop=True)

            def load_w(sb, kq):
                """One W k-quad [128, 4, 1024] of superblock sb (1MB DMA,
                scalar ring)."""
                w = wp.tile([P, 4, SBW], bf16, name=f"w_{sb}_{kq}", tag="w")
                nc.scalar.dma_start(
                    out=w[:],
                    in_=wT[:, kq * 4:(kq + 1) * 4,
                           sb * SBW:(sb + 1) * SBW],
                )
                return w

            # Startup-critical tiles first. x's q0 span is loaded in four
            # k-quads [128, 4, 512] (usable after ~0.5MB each) interleaved
            # with W sb0's four 1MB k-quads on the other ring; the
            # remaining q-spans are one 2MB DMA each.
            nxq = nxf + (1 if xrem else 0)
            xt = [None] * nxq            # q >= 1: [128, KO, span] tiles
            xq0 = [None] * (KO // 4)     # q == 0: per-k-quad tiles
            wq = {0: [None] * (KO // 4)}
            for kq in range(KO // 4):
                t = xq0p.tile([P, 4, XTW], bf16, name=f"x0_{kq}", tag="x0")
                nc.sync.dma_start(
                    out=t[:], in_=xT[:, kq * 4:(kq + 1) * 4, 0:XTW])
                xq0[kq] = t
                wq[0][kq] = load_w(0, kq)
            for q in range(1, nxq):
                w = XTW if q < nxf else xrem
                pool, tag = (xa, "x") if q < nxf else (xr, "xr")
                t = pool.tile([P, KO, w], bf16, name=f"x_{q}", tag=tag)
                nc.sync.dma_start(
                    out=t[:], in_=xT[:, :, q * XTW:q * XTW + w])
                xt[q] = t

            def xslice(k, m):
                q, r = divmod(m, XTW // P)
                if q == 0:
                    return xq0[k // 4][:, k % 4, r * P:(r + 1) * P]
                return xt[q][:, k, r * P:(r + 1) * P]

            cnt = 0
            pend = []  # deferred store triggers: (ot, m, sb, n2|None)

            def emit_store(ot, m, sb, n2=None):
                nonlocal cnt
                if n2 is not None:
                    # Final-drain halves: independent tiles on both rings.
                    ring = nc.sync if n2 == 0 else nc.scalar
                    n0 = sb * SBW + n2 * NCHUNK
                    ring.dma_start(
                        out=out[m * P:(m + 1) * P, n0:n0 + NCHUNK],
                        in_=ot[:],
                    )
                else:
                    ring = nc.sync if cnt % 2 == 0 else nc.scalar
                    ring.dma_start(
                        out=out[m * P:(m + 1) * P, sb * SBW:(sb + 1) * SBW],
                        in_=ot[:],
                    )
                cnt += 1

            for sb in range(NSB):
                wts = wq.pop(sb)
                for bi, (m0, bm) in enumerate(blocks):
                    ps = {}
                    for mi in range(bm):
                        for n2 in range(2):
                            ps[mi, n2] = pp.tile(
                                [P, NCHUNK], f32,
                                name=f"ps_{sb}_{m0 + mi}_{n2}", tag="ps",
                            )
                    for k in range(KO):
                        for mi in range(bm):
                            lhsT = xslice(k, m0 + mi)
                            for n2 in range(2):
                                nc.tensor.matmul(
                                    ps[mi, n2][:], lhsT,
                                    wts[k // 4][:, k % 4,
                                                n2 * NCHUNK:(n2 + 1) * NCHUNK],
                                    start=(k == 0), stop=(k == KO - 1),
                                )
                    last = (sb == NSB - 1 and bi == len(blocks) - 1)
                    for mi in range(bm):
                        m = m0 + mi
                        if last and mi == bm - 1:
                            # Final m-tile: two independent copy+store
                            # halves so store 0 overlaps copy 1.
                            for n2 in range(2):
                                ot = op.tile([P, NCHUNK], f32,
                                             name=f"o2_{n2}", tag="o2",
                                             bufs=2)
                                nc.vector.tensor_copy(ot[:], ps[mi, n2][:])
                                pend.append((ot, m, sb, n2))
                            continue
                        # Both 512-chunks staged into one [128,1024] tile by
                        # DVE (an engine nothing else queues on), one store.
                        ot = op.tile([P, SBW], f32, name=f"o_{sb}_{m}", tag="o")
                        for n2 in range(2):
                            nc.vector.tensor_copy(
                                ot[:, n2 * NCHUNK:(n2 + 1) * NCHUNK],
                                ps[mi, n2][:])
                        # Store triggers lag two blocks behind: HWDGE
                        # completion-semaphore lanes are shared by loads and
                        # stores, so early store receipts would pace the x/W
                        # tile waits of the first blocks.
                        pend.append((ot, m, sb, None))
                    while len(pend) > 2 * MBLK:
                        emit_store(*pend.pop(0))
                    if sb + 1 < NSB:
                        # Prefetch next superblock's W one 1MB quad every
                        # other block (no trigger bursts on the scalar ring).
                        nxt = wq.setdefault(sb + 1, [])
                        want = (bi // 2 + 1 if bi % 2 == 0 else len(nxt))
                        if bi == len(blocks) - 1:
                            want = KO // 4
                        while len(nxt) < min(want, KO // 4):
                            nxt.append(load_w(sb + 1, len(nxt)))
            while pend:
                emit_store(*pend.pop(0))
    nc.compile()
    return nc


def _plan(counts):
    """Allocate 8 cores to experts proportionally (largest remainder),
    then split each expert's token range into per-core contiguous spans.
    Returns (spans, t_max): spans[c] = (expert, start, length)."""
    total = int(counts.sum())
    ne = len(counts)
    active = [e for e in range(ne) if counts[e] > 0]
    quota = {e: counts[e] * NCORES / total for e in active}
    alloc = {e: max(1, int(quota[e])) for e in active}
    while sum(alloc.values()) > NCORES:  # too many mins; shrink largest
        shrinkable = [e for e in active if alloc[e] > 1]
        e = max(shrinkable, key=lambda e: alloc[e] - quota[e])
        alloc[e] -= 1
    rema = sorted(active, key=lambda e: quota[e] - alloc[e], reverse=True)
    i = 0
    while sum(alloc.values()) < NCORES:
        alloc[rema[i % len(rema)]] += 1
        i += 1
    spans = []
    starts = np.concatenate([[0], np.cumsum(counts)])
    for e in active:
        k = alloc[e]
        base, extra = divmod(int(counts[e]), k)
        off = int(starts[e])
        for j in range(k):
            ln = base + (1 if j < extra else 0)
            spans.append((e, off, ln))
            off += ln
    t_max = max(ln for _, _, ln in spans)
    t_max = max(P, -(-t_max // P) * P)
    return spans, t_max


def _plan_fixed(counts, mm_sorted, T):
    """Fixed equal shards: core c gets tokens [c*L, (c+1)*L) regardless of
    expert boundaries, with L a multiple of 128 (zero padding only on the
    last core when T doesn't divide). Each core computes with the weight of
    its majority expert; the few boundary tokens whose expert differs are
    recomputed exactly on the host afterwards. Returns (spans, t_max,
    n_mismatch) with spans[c] = (expert, start, length)."""
    L = -(-T // (NCORES * P)) * P
    spans = []
    n_mis = 0
    for c in range(NCORES):
        off = c * L
        ln = max(0, min(L, T - off))
        ids = mm_sorted[off:off + ln]
        e = int(np.bincount(ids, minlength=len(counts)).argmax()) if ln else 0
        n_mis += int((ids != e).sum())
        spans.append((e, off, ln))
    return spans, L, n_mis


def _plan_auto(counts, mm_sorted, T):
    """Fixed equal shards when the host boundary-patch is small (the
    normal case for near-uniform expert counts), else the padded
    expert-pure plan. Returns (spans, t_max, n_mismatch)."""
    spans, t_max, n_mis = _plan_fixed(counts, mm_sorted, T)
    if n_mis > T // 8:
        spans, t_max = _plan(counts)
        n_mis = 0
    return spans, t_max, n_mis


def _make_in_maps(x, W, spans, t_max):
    """Per-core input dicts, both pre-arranged as [128, k-tile, col] bf16
    so the device can pull many k-tiles per (large, line-rate) DMA."""
    import ml_dtypes

    bf16 = ml_dtypes.bfloat16
    xb = np.asarray(x, dtype=np.float32).astype(bf16)
    wTs = {}
    in_maps = []
    for e, off, ln in spans:
        if e not in wTs:
            wT = np.asarray(W[e], dtype=np.float32).T.astype(bf16)
            wTs[e] = np.ascontiguousarray(
                wT.reshape(KO, P, QKV_OUT).transpose(1, 0, 2))
        xTp = np.zeros((P, KO, t_max), dtype=bf16)
        xTp[:, :, :ln] = (
            xb[off:off + ln].T.reshape(KO, P, ln).transpose(1, 0, 2))
        in_maps.append({"xT": xTp, "wT": wTs[e]})
    return in_maps


def _runner(mt: int):
    """Compiled 8-core executor for the mt-tile module, cached so repeat
    kernel() calls skip jax retracing. Mirrors bass2jax.run_bass_via_pjrt's
    multi-core path (concat per-core inputs on axis 0 + shard_map)."""
    import jax
    import jax.numpy as jnp
    from jax.sharding import Mesh, PartitionSpec
    from jax.experimental.shard_map import shard_map
    from concourse import bass2jax, mybir as mb

    nc = _build(mt)
    bass2jax.install_neuronx_cc_hook()

    part_name = nc.partition_id_tensor.name if nc.partition_id_tensor else None
    in_names, out_names, out_avals = [], [], []
    for alloc in nc.m.functions[0].allocations:
        if not isinstance(alloc, mb.MemoryLocationSet):
            continue
        name = alloc.memorylocations[0].name
        if alloc.kind == "ExternalInput":
            if name != part_name:
                in_names.append(name)
        elif alloc.kind == "ExternalOutput":
            out_names.append(name)
            out_avals.append(
                jax.core.ShapedArray(tuple(alloc.tensor_shape),
                                     mb.dt.np(alloc.dtype)))
    n_params = len(in_names)
    n_outs = len(out_names)
    bind_names = in_names + out_names + ([part_name] if part_name else [])

    def _body(*args):
        operands = list(args)
        if part_name:
            operands.append(bass2jax.partition_id_tensor())
        outs = bass2jax._bass_exec_p.bind(
            *operands,
            out_avals=tuple(out_avals),
            in_names=tuple(bind_names),
            out_names=tuple(out_names),
            lowering_input_output_aliases=(),
            sim_require_finite=True,
            sim_require_nnan=True,
            nc=nc,
        )
        return tuple(outs)

    devices = jax.devices()[:NCORES]
    mesh = Mesh(np.asarray(devices), ("core",))
    sharded = jax.jit(
        shard_map(_body, mesh=mesh,
                  in_specs=(PartitionSpec("core"),) * (n_params + n_outs),
                  out_specs=(PartitionSpec("core"),) * n_outs,
                  check_rep=False),
        donate_argnums=tuple(range(n_params, n_params + n_outs)),
        keep_unused=True,
    )

    def run(in_maps):
        concat_in = [
            np.concatenate([m[name] for m in in_maps], axis=0)
            for name in in_names
        ]
        zeros = [np.zeros((NCORES * a.shape[0], *a.shape[1:]), a.dtype)
                 for a in out_avals]
        outs = sharded(*concat_in, *zeros)
        return [
            {name: np.asarray(outs[i]).reshape(NCORES, *out_avals[i].shape)[c]
             for i, name in enumerate(out_names)}
            for c in range(NCORES)
        ]

    return run


def kernel(x, W, modality_mapping):
    x = np.ascontiguousarray(np.asarray(x, dtype=np.float32))
    W = np.asarray(W, dtype=np.float32)
    mm = np.asarray(modality_mapping)

    perm = None
    if np.any(np.diff(mm) < 0):  # insurance: tokens not pre-sorted
        perm = np.argsort(mm, kind="stable")
        x = x[perm]
        mm = mm[perm]

    T = x.shape[0]
    E = W.shape[0]
    counts = np.bincount(mm.astype(np.int64), minlength=E)
    spans, t_max, n_mis = _plan_auto(counts, mm, T)
    mt = t_max // P

    if mt not in _cache:
        _cache[mt] = _runner(mt)
    run = _cache[mt]

    in_maps = _make_in_maps(x, W, spans, t_max)
    results = run(in_maps)

    out = np.empty((T, QKV_OUT), dtype=np.float32)
    ec = np.empty(T, dtype=np.int64)
    for c, (e, off, ln) in enumerate(spans):
        out[off:off + ln] = results[c]["out"][:ln]
        ec[off:off + ln] = e
    if n_mis:
        # Recompute boundary tokens (expert != core majority) exactly.
        bad = np.nonzero(mm.astype(np.int64) != ec)[0]
        for e in np.unique(mm[bad]):
            rows = bad[mm[bad] == e]
            out[rows] = x[rows] @ W[int(e)].T
    if perm is not None:
        inv = np.empty_like(perm)
        inv[perm] = np.arange(T)
        out = out[inv]
    return out


# revision 34
# speedup vs baseline: 1.0239x; 1.0239x over previous
"""MoE QKV parallel linear for Trainium2, 8 NeuronCores.

Problem: out[t] = x[t] @ W[id[t]].T with x [16384, 2048] f32,
W [4, 3072, 2048] f32, id sorted int32 (tokens pre-grouped by expert).

Sharding: data-parallel over tokens, fixed equal shards. Core c gets
tokens [c*2048, (c+1)*2048) and computes a dense [2048, 2048] @
[2048, 3072] matmul against its majority expert's weight (no padding, 16
m-tiles exactly); the few boundary tokens whose expert differs (~100 for
near-uniform counts) are recomputed exactly on the host and patched in.
Falls back to padded expert-pure shards under heavy skew. Host transposes
x-shards and W and casts both to bf16 (absmax-rel error of the
bf16 x bf16 / f32-accum matmul is ~2.2e-3, vs the 2e-2 gate).

Device kernel (per core), all sized to hardware-measured costs:
- bf16 operands: matmuls lower to LDWEIGHTS+MATMUL pairs the PE reorder
  window pipelines (f32r matmuls stay self-loading and pay the weight
  load serially, ~25% slower).
- x^T fully SBUF-resident as [128, 512] tiles; W^T streamed once in three
  1024-wide column superblocks, next superblock prefetched 2 tiles/block.
- m-blocks of 2 token-tiles: 4 PSUM banks live per block, the other 4
  spare, so PSUM recycling never stalls the accumulation (k-outer,
  start=k0/stop=k15, f32 accumulation).
- All PSUM->SBUF copies on DVE (its queue has nothing else; HWDGE
  dma_start triggers cost ~0.6us of the issuing engine and head-block it),
  two 512-chunks staged into one [128, 1024] tile per store.
- Store triggers lag two blocks so early completion-semaphore lanes only
  pace loads; startup x/W tiles interleave across both HWDGE rings; 12
  zero matmuls pre-warm the HAM clock gate during the initial DMA wait.
"""

import numpy as np

import concourse.bacc as bacc
import concourse.mybir as mybir
import concourse.tile as tile

NCORES = 8
HIDDEN = 2048
QKV_OUT = 3072
P = 128
KO = HIDDEN // P          # 16 contraction tiles
NCHUNK = 512              # PSUM free dim per matmul (fp32 max)
SBW = 1024                # W column superblock width
NSB = QKV_OUT // SBW      # 3 superblocks
MBLK = 2                  # m-tiles per block (4 PSUM banks live, 4 spare)
XTW = 4 * P               # x resident tile width (tokens)

_cache: dict = {}


def _blocks(mt: int):
    """Partition mt m-tiles into blocks of MBLK, ending with two 1-tile
    blocks so the final PSUM drain (copies + last store) is short."""
    out = []
    m0 = 0
    while mt - m0 > 2:
        out.append((m0, MBLK))
        m0 += MBLK
    while m0 < mt:
        out.append((m0, 1))
        m0 += 1
    return out


def _build(mt: int):
    """Bass module for one core: out[mt*128, 3072] = xT.T @ wT (bf16 in)."""
    nc = bacc.Bacc("TRN2", target_bir_lowering=False, debug=False)
    tmax = mt * P
    bf16 = mybir.dt.bfloat16
    f32 = mybir.dt.float32

    xT = nc.dram_tensor("xT", [HIDDEN, tmax], bf16, kind="ExternalInput")
    wT = nc.dram_tensor("wT", [HIDDEN, QKV_OUT], bf16, kind="ExternalInput")
    out = nc.dram_tensor("out", [tmax, QKV_OUT], f32, kind="ExternalOutput")

    blocks = _blocks(mt)
    nxf, xrem = divmod(tmax, XTW)  # full x tiles per k-row + ragged tail

    with tile.TileContext(nc) as tc:
        with (
            tc.tile_pool(name="xa", bufs=max(1, KO * nxf)) as xa,
            tc.tile_pool(name="xr", bufs=max(1, KO * (1 if xrem else 0))) as xr,
            tc.tile_pool(name="wp", bufs=2 * KO) as wp,
            tc.tile_pool(name="pp", bufs=8, space="PSUM") as pp,
            tc.tile_pool(name="op", bufs=8) as op,
        ):
            # Resident x^T: tiles[k][q] of [128, 512] bf16 on the sync ring,
            # q-major so arrival matches block consumption order.
            # PE warmup: the HAM clock gate starts at K=4/8 (1.2 GHz) and
            # needs ~3.4us of sustained matmul activity to release. The PE
            # is idle for the first ~10us waiting on DMA anyway, so run
            # dummy matmuls on zeroed scratch to enter the kernel warm.
            wl = wp.tile([P, P], bf16, name="warm_l", tag="warm_l", bufs=1)
            wr = wp.tile([P, NCHUNK], bf16, name="warm_r", tag="warm_r",
                         bufs=1)
            nc.vector.memset(wl[:], 0.0)
            nc.vector.memset(wr[:], 0.0)
            psw = pp.tile([P, NCHUNK], f32, name="warm_ps", tag="ps")
            for _ in range(12):
                nc.tensor.matmul(psw[:], wl[:], wr[:], start=True, stop=True)

            def load_w(sb, ks, eng=None):
                """W k-tiles [ks] of superblock sb (scalar ring default)."""
                wts = []
                for k in ks:
                    w = wp.tile([P, SBW], bf16, name=f"w_{sb}_{k}", tag="w")
                    e = eng(k) if eng else nc.scalar
                    e.dma_start(
                        out=w[:],
                        in_=wT[k * P:(k + 1) * P, sb * SBW:(sb + 1) * SBW],
                    )
                    wts.append(w)
                return wts

            # Startup-critical tiles first — x q0 and W sb0 interleaved
            # per k on opposite HWDGE rings, so neither ring's trigger
            # pacing (~0.6us per dma_start) serializes block 0's stream.
            nxq = nxf + (1 if xrem else 0)
            xt = [[None] * nxq for _ in range(KO)]

            def load_x(q, ks):
                w = XTW if q < nxf else xrem
                for k in ks:
                    pool, tag = (xa, "x") if q < nxf else (xr, "xr")
                    t = pool.tile([P, w], bf16, name=f"x_{k}_{q}", tag=tag)
                    eng = nc.sync if (k + q) % 2 == 0 else nc.scalar
                    eng.dma_start(
                        out=t[:],
                        in_=xT[k * P:(k + 1) * P, q * XTW:q * XTW + w],
                    )
                    xt[k][q] = t

            wts0 = []
            for k in range(KO):
                load_x(0, [k])
                wts0.extend(load_w(0, [k],
                                   eng=lambda kk: nc.scalar if kk % 2 == 0
                                   else nc.sync))
            for q in range(1, nxq):
                load_x(q, range(KO))

            def xslice(k, m):
                q, r = divmod(m, XTW // P)
                return xt[k][q][:, r * P:(r + 1) * P]

            # Next superblock's W is prefetched a couple of tiles per block:
            # a 16-tile burst clogs the scalar engine queue (each HWDGE
            # trigger costs ~0.6us engine time + lane-pacing waits).
            wpre = -(-KO // len(blocks))
            wq = {0: wts0}
            cnt = 0
            pend = []  # deferred store triggers: (ot, m, sb, n2|None)

            def emit_store(ot, m, sb, n2=None):
                nonlocal cnt
                if n2 is not None:
                    # Final-drain halves: independent tiles on both rings.
                    ring = nc.sync if n2 == 0 else nc.scalar
                    n0 = sb * SBW + n2 * NCHUNK
                    ring.dma_start(
                        out=out[m * P:(m + 1) * P, n0:n0 + NCHUNK],
                        in_=ot[:],
                    )
                else:
                    ring = nc.sync if cnt % 2 == 0 else nc.scalar
                    ring.dma_start(
                        out=out[m * P:(m + 1) * P, sb * SBW:(sb + 1) * SBW],
                        in_=ot[:],
                    )
                cnt += 1

            for sb in range(NSB):
                wts = wq.pop(sb)
                for bi, (m0, bm) in enumerate(blocks):
                    ps = {}
                    for mi in range(bm):
                        for n2 in range(2):
                            ps[mi, n2] = pp.tile(
                                [P, NCHUNK], f32,
                                name=f"ps_{sb}_{m0 + mi}_{n2}", tag="ps",
                            )
                    for k in range(KO):
                        for mi in range(bm):
                            lhsT = xslice(k, m0 + mi)
                            for n2 in range(2):
                                nc.tensor.matmul(
                                    ps[mi, n2][:], lhsT,
                                    wts[k][:, n2 * NCHUNK:(n2 + 1) * NCHUNK],
                                    start=(k == 0), stop=(k == KO - 1),
                                )
                    last = (sb == NSB - 1 and bi == len(blocks) - 1)
                    for mi in range(bm):
                        m = m0 + mi
                        if last and mi == bm - 1:
                            # Final m-tile: two independent copy+store
                            # halves so store 0 overlaps copy 1.
                            for n2 in range(2):
                                ot = op.tile([P, NCHUNK], f32,
                                             name=f"o2_{n2}", tag="o2",
                                             bufs=2)
                                nc.vector.tensor_copy(ot[:], ps[mi, n2][:])
                                pend.append((ot, m, sb, n2))
                            continue
                        # Both 512-chunks staged into one [128,1024] tile by
                        # DVE (an engine nothing else queues on), one store.
                        ot = op.tile([P, SBW], f32, name=f"o_{sb}_{m}", tag="o")
                        for n2 in range(2):
                            nc.vector.tensor_copy(
                                ot[:, n2 * NCHUNK:(n2 + 1) * NCHUNK],
                                ps[mi, n2][:])
                        # Store triggers lag two blocks behind: HWDGE
                        # completion-semaphore lanes are shared by loads and
                        # stores, so early store receipts would pace the x/W
                        # tile waits of the first blocks.
                        pend.append((ot, m, sb, None))
                    while len(pend) > 2 * MBLK:
                        emit_store(*pend.pop(0))
                    if sb + 1 < NSB:
                        ks = range(wpre * bi, min(wpre * (bi + 1), KO))
                        if bi == len(blocks) - 1:
                            ks = range(wpre * bi, KO)
                        if ks:
                            wq.setdefault(sb + 1, []).extend(load_w(sb + 1, ks))
            while pend:
                emit_store(*pend.pop(0))
    nc.compile()
    return nc


def _plan(counts):
    """Allocate 8 cores to experts proportionally (largest remainder),
    then split each expert's token range into per-core contiguous spans.
    Returns (spans, t_max): spans[c] = (expert, start, length)."""
    total = int(counts.sum())
    ne = len(counts)
    active = [e for e in range(ne) if counts[e] > 0]
    quota = {e: counts[e] * NCORES / total for e in active}
    alloc = {e: max(1, int(quota[e])) for e in active}
    while sum(alloc.values()) > NCORES:  # too many mins; shrink largest
        shrinkable = [e for e in active if alloc[e] > 1]
        e = max(shrinkable, key=lambda e: alloc[e] - quota[e])
        alloc[e] -= 1
    rema = sorted(active, key=lambda e: quota[e] - alloc[e], reverse=True)
    i = 0
    while sum(alloc.values()) < NCORES:
        alloc[rema[i % len(rema)]] += 1
        i += 1
    spans = []
    starts = np.concatenate([[0], np.cumsum(counts)])
    for e in active:
        k = alloc[e]
        base, extra = divmod(int(counts[e]), k)
        off = int(starts[e])
        for j in range(k):
            ln = base + (1 if j < extra else 0)
            spans.append((e, off, ln))
            off += ln
    t_max = max(ln for _, _, ln in spans)
    t_max = max(P, -(-t_max // P) * P)
    return spans, t_max


def _plan_fixed(counts, mm_sorted, T):
    """Fixed equal shards: core c gets tokens [c*L, (c+1)*L) regardless of
    expert boundaries, with L a multiple of 128 (zero padding only on the
    last core when T doesn't divide). Each core computes with the weight of
    its majority expert; the few boundary tokens whose expert differs are
    recomputed exactly on the host afterwards. Returns (spans, t_max,
    n_mismatch) with spans[c] = (expert, start, length)."""
    L = -(-T // (NCORES * P)) * P
    spans = []
    n_mis = 0
    for c in range(NCORES):
        off = c * L
        ln = max(0, min(L, T - off))
        ids = mm_sorted[off:off + ln]
        e = int(np.bincount(ids, minlength=len(counts)).argmax()) if ln else 0
        n_mis += int((ids != e).sum())
        spans.append((e, off, ln))
    return spans, L, n_mis


def _plan_auto(counts, mm_sorted, T):
    """Fixed equal shards when the host boundary-patch is small (the
    normal case for near-uniform expert counts), else the padded
    expert-pure plan. Returns (spans, t_max, n_mismatch)."""
    spans, t_max, n_mis = _plan_fixed(counts, mm_sorted, T)
    if n_mis > T // 8:
        spans, t_max = _plan(counts)
        n_mis = 0
    return spans, t_max, n_mis


def _make_in_maps(x, W, spans, t_max):
    """Per-core input dicts: bf16 x^T shard (zero-padded) + bf16 W^T."""
    import ml_dtypes

    bf16 = ml_dtypes.bfloat16
    xb = np.asarray(x, dtype=np.float32).astype(bf16)
    wTs = {}
    in_maps = []
    for e, off, ln in spans:
        if e not in wTs:
            wTs[e] = np.ascontiguousarray(
                np.asarray(W[e], dtype=np.float32).T.astype(bf16))
        xTp = np.zeros((HIDDEN, t_max), dtype=bf16)
        xTp[:, :ln] = xb[off:off + ln].T
        in_maps.append({"xT": xTp, "wT": wTs[e]})
    return in_maps


def _runner(mt: int):
    """Compiled 8-core executor for the mt-tile module, cached so repeat
    kernel() calls skip jax retracing. Mirrors bass2jax.run_bass_via_pjrt's
    multi-core path (concat per-core inputs on axis 0 + shard_map)."""
    import jax
    import jax.numpy as jnp
    from jax.sharding import Mesh, PartitionSpec
    from jax.experimental.shard_map import shard_map
    from concourse import bass2jax, mybir as mb

    nc = _build(mt)
    bass2jax.install_neuronx_cc_hook()

    part_name = nc.partition_id_tensor.name if nc.partition_id_tensor else None
    in_names, out_names, out_avals = [], [], []
    for alloc in nc.m.functions[0].allocations:
        if not isinstance(alloc, mb.MemoryLocationSet):
            continue
        name = alloc.memorylocations[0].name
        if alloc.kind == "ExternalInput":
            if name != part_name:
                in_names.append(name)
        elif alloc.kind == "ExternalOutput":
            out_names.append(name)
            out_avals.append(
                jax.core.ShapedArray(tuple(alloc.tensor_shape),
                                     mb.dt.np(alloc.dtype)))
    n_params = len(in_names)
    n_outs = len(out_names)
    bind_names = in_names + out_names + ([part_name] if part_name else [])

    def _body(*args):
        operands = list(args)
        if part_name:
            operands.append(bass2jax.partition_id_tensor())
        outs = bass2jax._bass_exec_p.bind(
            *operands,
            out_avals=tuple(out_avals),
            in_names=tuple(bind_names),
            out_names=tuple(out_names),
            lowering_input_output_aliases=(),
            sim_require_finite=True,
            sim_require_nnan=True,
            nc=nc,
        )
        return tuple(outs)

    devices = jax.devices()[:NCORES]
    mesh = Mesh(np.asarray(devices), ("core",))
    sharded = jax.jit(
        shard_map(_body, mesh=mesh,
                  in_specs=(PartitionSpec("core"),) * (n_params + n_outs),
                  out_specs=(PartitionSpec("core"),) * n_outs,
                  check_rep=False),
        donate_argnums=tuple(range(n_params, n_params + n_outs)),
        keep_unused=True,
    )

    def run(in_maps):
        concat_in = [
            np.concatenate([m[name] for m in in_maps], axis=0)
            for name in in_names
        ]
        zeros = [np.zeros((NCORES * a.shape[0], *a.shape[1:]), a.dtype)
                 for a in out_avals]
        outs = sharded(*concat_in, *zeros)
        return [
            {name: np.asarray(outs[i]).reshape(NCORES, *out_avals[i].shape)[c]
             for i, name in enumerate(out_names)}
            for c in range(NCORES)
        ]

    return run


def kernel(x, W, modality_mapping):
    x = np.ascontiguousarray(np.asarray(x, dtype=np.float32))
    W = np.asarray(W, dtype=np.float32)
    mm = np.asarray(modality_mapping)

    perm = None
    if np.any(np.diff(mm) < 0):  # insurance: tokens not pre-sorted
        perm = np.argsort(mm, kind="stable")
        x = x[perm]
        mm = mm[perm]

    T = x.shape[0]
    E = W.shape[0]
    counts = np.bincount(mm.astype(np.int64), minlength=E)
    spans, t_max, n_mis = _plan_auto(counts, mm, T)
    mt = t_max // P

    if mt not in _cache:
        _cache[mt] = _runner(mt)
    run = _cache[mt]

    in_maps = _make_in_maps(x, W, spans, t_max)
    results = run(in_maps)

    out = np.empty((T, QKV_OUT), dtype=np.float32)
    ec = np.empty(T, dtype=np.int64)
    for c, (e, off, ln) in enumerate(spans):
        out[off:off + ln] = results[c]["out"][:ln]
        ec[off:off + ln] = e
    if n_mis:
        # Recompute boundary tokens (expert != core majority) exactly.
        bad = np.nonzero(mm.astype(np.int64) != ec)[0]
        for e in np.unique(mm[bad]):
            rows = bad[mm[bad] == e]
            out[rows] = x[rows] @ W[int(e)].T
    if perm is not None:
        inv = np.empty_like(perm)
        inv[perm] = np.arange(T)
        out = out[inv]
    return out


# revision 36
# speedup vs baseline: 1.0407x; 1.0164x over previous
"""MoE QKV parallel linear for Trainium2, 8 NeuronCores.

Problem: out[t] = x[t] @ W[id[t]].T with x [16384, 2048] f32,
W [4, 3072, 2048] f32, id sorted int32 (tokens pre-grouped by expert).

Sharding: data-parallel over tokens, fixed equal shards. Core c gets
tokens [c*2048, (c+1)*2048) and computes a dense [2048, 2048] @
[2048, 3072] matmul against its majority expert's weight (no padding, 16
m-tiles exactly); the few boundary tokens whose expert differs (~100 for
near-uniform counts) are recomputed exactly on the host and patched in.
Falls back to padded expert-pure shards under heavy skew. Host transposes
x-shards and W and casts both to bf16 (absmax-rel error of the
bf16 x bf16 / f32-accum matmul is ~2.2e-3, vs the 2e-2 gate).

Device kernel (per core), all sized to hardware-measured costs:
- bf16 operands: matmuls lower to LDWEIGHTS+MATMUL pairs the PE reorder
  window pipelines (f32r matmuls stay self-loading and pay the weight
  load serially, ~25% slower).
- x^T fully SBUF-resident as [128, 512] tiles; W^T streamed once in three
  1024-wide column superblocks, next superblock prefetched 2 tiles/block.
- m-blocks of 2 token-tiles: 4 PSUM banks live per block, the other 4
  spare, so PSUM recycling never stalls the accumulation (k-outer,
  start=k0/stop=k15, f32 accumulation).
- All PSUM->SBUF copies on DVE (its queue has nothing else; HWDGE
  dma_start triggers cost ~0.6us of the issuing engine and head-block it),
  two 512-chunks staged into one [128, 1024] tile per store.
- Store triggers lag two blocks so early completion-semaphore lanes only
  pace loads; startup x/W tiles interleave across both HWDGE rings; 12
  zero matmuls pre-warm the HAM clock gate during the initial DMA wait.
"""

import numpy as np

import concourse.bacc as bacc
import concourse.mybir as mybir
import concourse.tile as tile

NCORES = 8
HIDDEN = 2048
QKV_OUT = 3072
P = 128
KO = HIDDEN // P          # 16 contraction tiles
NCHUNK = 512              # PSUM free dim per matmul (fp32 max)
SBW = 1024                # W column superblock width
NSB = QKV_OUT // SBW      # 3 superblocks
MBLK = 2                  # m-tiles per block (4 PSUM banks live, 4 spare)
XTW = 4 * P               # x resident tile width (tokens)

_cache: dict = {}


def _blocks(mt: int):
    """Partition mt m-tiles into blocks of MBLK, with a fatter FIRST block
    (3 m-tiles: its k-loop consumes x/W tiles at ~300 GB/s, matching the
    ramping HBM delivery, where a 2-tile block would outrun it and stall)
    and 1-tile blocks at the end so the final PSUM drain is short."""
    out = []
    m0 = 0
    if mt > 4:
        out.append((0, 3))
        m0 = 3
    while mt - m0 > 2:
        out.append((m0, MBLK))
        m0 += MBLK
    while m0 < mt:
        out.append((m0, 1))
        m0 += 1
    return out


def _build(mt: int):
    """Bass module for one core: out[mt*128, 3072] = xT.T @ wT (bf16 in)."""
    nc = bacc.Bacc("TRN2", target_bir_lowering=False, debug=False)
    tmax = mt * P
    bf16 = mybir.dt.bfloat16
    f32 = mybir.dt.float32

    xT = nc.dram_tensor("xT", [HIDDEN, tmax], bf16, kind="ExternalInput")
    wT = nc.dram_tensor("wT", [HIDDEN, QKV_OUT], bf16, kind="ExternalInput")
    out = nc.dram_tensor("out", [tmax, QKV_OUT], f32, kind="ExternalOutput")

    blocks = _blocks(mt)
    nxf, xrem = divmod(tmax, XTW)  # full x tiles per k-row + ragged tail

    with tile.TileContext(nc) as tc:
        with (
            tc.tile_pool(name="xa", bufs=max(1, KO * nxf)) as xa,
            tc.tile_pool(name="xr", bufs=max(1, KO * (1 if xrem else 0))) as xr,
            tc.tile_pool(name="wp", bufs=2 * KO) as wp,
            tc.tile_pool(name="pp", bufs=8, space="PSUM") as pp,
            tc.tile_pool(name="op", bufs=8) as op,
        ):
            # Resident x^T: tiles[k][q] of [128, 512] bf16 on the sync ring,
            # q-major so arrival matches block consumption order.
            # PE warmup: the HAM clock gate starts at K=4/8 (1.2 GHz) and
            # needs ~3.4us of sustained matmul activity to release. The PE
            # is idle for the first ~10us waiting on DMA anyway, so run
            # dummy matmuls on zeroed scratch to enter the kernel warm.
            wl = wp.tile([P, P], bf16, name="warm_l", tag="warm_l", bufs=1)
            wr = wp.tile([P, NCHUNK], bf16, name="warm_r", tag="warm_r",
                         bufs=1)
            nc.vector.memset(wl[:], 0.0)
            nc.vector.memset(wr[:], 0.0)
            psw = pp.tile([P, NCHUNK], f32, name="warm_ps", tag="ps")
            for _ in range(12):
                nc.tensor.matmul(psw[:], wl[:], wr[:], start=True, stop=True)

            def load_w(sb, ks, eng=None):
                """W k-tiles [ks] of superblock sb (scalar ring default)."""
                wts = []
                for k in ks:
                    w = wp.tile([P, SBW], bf16, name=f"w_{sb}_{k}", tag="w")
                    e = eng(k) if eng else nc.scalar
                    e.dma_start(
                        out=w[:],
                        in_=wT[k * P:(k + 1) * P, sb * SBW:(sb + 1) * SBW],
                    )
                    wts.append(w)
                return wts

            # Startup-critical tiles first — x q0 and W sb0 interleaved
            # per k on opposite HWDGE rings, so neither ring's trigger
            # pacing (~0.6us per dma_start) serializes block 0's stream.
            nxq = nxf + (1 if xrem else 0)
            xt = [[None] * nxq for _ in range(KO)]

            def load_x(q, ks):
                w = XTW if q < nxf else xrem
                for k in ks:
                    pool, tag = (xa, "x") if q < nxf else (xr, "xr")
                    t = pool.tile([P, w], bf16, name=f"x_{k}_{q}", tag=tag)
                    eng = nc.sync if (k + q) % 2 == 0 else nc.scalar
                    eng.dma_start(
                        out=t[:],
                        in_=xT[k * P:(k + 1) * P, q * XTW:q * XTW + w],
                    )
                    xt[k][q] = t

            wts0 = []
            for k in range(KO):
                load_x(0, [k])
                wts0.extend(load_w(0, [k],
                                   eng=lambda kk: nc.scalar if kk % 2 == 0
                                   else nc.sync))
            for q in range(1, nxq):
                load_x(q, range(KO))

            def xslice(k, m):
                q, r = divmod(m, XTW // P)
                return xt[k][q][:, r * P:(r + 1) * P]

            # Next superblock's W is prefetched a couple of tiles per block:
            # a 16-tile burst clogs the scalar engine queue (each HWDGE
            # trigger costs ~0.6us engine time + lane-pacing waits).
            wpre = -(-KO // len(blocks))
            wq = {0: wts0}
            cnt = 0
            pend = []  # deferred store triggers: (ot, m, sb)

            def emit_store(ot, m, sb, split=False):
                nonlocal cnt
                if split:
                    # Final drain: halves on both rings in parallel.
                    for n2 in range(2):
                        ring = nc.sync if n2 == 0 else nc.scalar
                        n0 = sb * SBW + n2 * NCHUNK
                        ring.dma_start(
                            out=out[m * P:(m + 1) * P, n0:n0 + NCHUNK],
                            in_=ot[:, n2 * NCHUNK:(n2 + 1) * NCHUNK],
                        )
                else:
                    ring = nc.sync if cnt % 2 == 0 else nc.scalar
                    ring.dma_start(
                        out=out[m * P:(m + 1) * P, sb * SBW:(sb + 1) * SBW],
                        in_=ot[:],
                    )
                cnt += 1

            for sb in range(NSB):
                wts = wq.pop(sb)
                for bi, (m0, bm) in enumerate(blocks):
                    ps = {}
                    for mi in range(bm):
                        for n2 in range(2):
                            ps[mi, n2] = pp.tile(
                                [P, NCHUNK], f32,
                                name=f"ps_{sb}_{m0 + mi}_{n2}", tag="ps",
                            )
                    for k in range(KO):
                        for mi in range(bm):
                            lhsT = xslice(k, m0 + mi)
                            for n2 in range(2):
                                nc.tensor.matmul(
                                    ps[mi, n2][:], lhsT,
                                    wts[k][:, n2 * NCHUNK:(n2 + 1) * NCHUNK],
                                    start=(k == 0), stop=(k == KO - 1),
                                )
                    for mi in range(bm):
                        m = m0 + mi
                        # Both 512-chunks staged into one [128,1024] tile by
                        # DVE (an engine nothing else queues on), one store.
                        ot = op.tile([P, SBW], f32, name=f"o_{sb}_{m}", tag="o")
                        for n2 in range(2):
                            nc.vector.tensor_copy(
                                ot[:, n2 * NCHUNK:(n2 + 1) * NCHUNK],
                                ps[mi, n2][:])
                        # Store triggers lag two blocks behind: HWDGE
                        # completion-semaphore lanes are shared by loads and
                        # stores, so early store receipts would pace the x/W
                        # tile waits of the first blocks.
                        pend.append((ot, m, sb))
                    while len(pend) > 2 * MBLK:
                        emit_store(*pend.pop(0))
                    if sb + 1 < NSB:
                        ks = range(wpre * bi, min(wpre * (bi + 1), KO))
                        if bi == len(blocks) - 1:
                            ks = range(wpre * bi, KO)
                        if ks:
                            wq.setdefault(sb + 1, []).extend(load_w(sb + 1, ks))
            while pend:
                emit_store(*pend.pop(0), split=(len(pend) == 0))
    nc.compile()
    return nc


def _plan(counts):
    """Allocate 8 cores to experts proportionally (largest remainder),
    then split each expert's token range into per-core contiguous spans.
    Returns (spans, t_max): spans[c] = (expert, start, length)."""
    total = int(counts.sum())
    ne = len(counts)
    active = [e for e in range(ne) if counts[e] > 0]
    quota = {e: counts[e] * NCORES / total for e in active}
    alloc = {e: max(1, int(quota[e])) for e in active}
    while sum(alloc.values()) > NCORES:  # too many mins; shrink largest
        shrinkable = [e for e in active if alloc[e] > 1]
        e = max(shrinkable, key=lambda e: alloc[e] - quota[e])
        alloc[e] -= 1
    rema = sorted(active, key=lambda e: quota[e] - alloc[e], reverse=True)
    i = 0
    while sum(alloc.values()) < NCORES:
        alloc[rema[i % len(rema)]] += 1
        i += 1
    spans = []
    starts = np.concatenate([[0], np.cumsum(counts)])
    for e in active:
        k = alloc[e]
        base, extra = divmod(int(counts[e]), k)
        off = int(starts[e])
        for j in range(k):
            ln = base + (1 if j < extra else 0)
            spans.append((e, off, ln))
            off += ln
    t_max = max(ln for _, _, ln in spans)
    t_max = max(P, -(-t_max // P) * P)
    return spans, t_max


def _plan_fixed(counts, mm_sorted, T):
    """Fixed equal shards: core c gets tokens [c*L, (c+1)*L) regardless of
    expert boundaries, with L a multiple of 128 (zero padding only on the
    last core when T doesn't divide). Each core computes with the weight of
    its majority expert; the few boundary tokens whose expert differs are
    recomputed exactly on the host afterwards. Returns (spans, t_max,
    n_mismatch) with spans[c] = (expert, start, length)."""
    L = -(-T // (NCORES * P)) * P
    spans = []
    n_mis = 0
    for c in range(NCORES):
        off = c * L
        ln = max(0, min(L, T - off))
        ids = mm_sorted[off:off + ln]
        e = int(np.bincount(ids, minlength=len(counts)).argmax()) if ln else 0
        n_mis += int((ids != e).sum())
        spans.append((e, off, ln))
    return spans, L, n_mis


def _plan_auto(counts, mm_sorted, T):
    """Fixed equal shards when the host boundary-patch is small (the
    normal case for near-uniform expert counts), else the padded
    expert-pure plan. Returns (spans, t_max, n_mismatch)."""
    spans, t_max, n_mis = _plan_fixed(counts, mm_sorted, T)
    if n_mis > T // 8:
        spans, t_max = _plan(counts)
        n_mis = 0
    return spans, t_max, n_mis


def _make_in_maps(x, W, spans, t_max):
    """Per-core input dicts: bf16 x^T shard (zero-padded) + bf16 W^T."""
    import ml_dtypes

    bf16 = ml_dtypes.bfloat16
    xb = np.asarray(x, dtype=np.float32).astype(bf16)
    wTs = {}
    in_maps = []
    for e, off, ln in spans:
        if e not in wTs:
            wTs[e] = np.ascontiguousarray(
                np.asarray(W[e], dtype=np.float32).T.astype(bf16))
        xTp = np.zeros((HIDDEN, t_max), dtype=bf16)
        xTp[:, :ln] = xb[off:off + ln].T
        in_maps.append({"xT": xTp, "wT": wTs[e]})
    return in_maps


def _runner(mt: int):
    """Compiled 8-core executor for the mt-tile module, cached so repeat
    kernel() calls skip jax retracing. Mirrors bass2jax.run_bass_via_pjrt's
    multi-core path (concat per-core inputs on axis 0 + shard_map)."""
    import jax
    import jax.numpy as jnp
    from jax.sharding import Mesh, PartitionSpec
    from jax.experimental.shard_map import shard_map
    from concourse import bass2jax, mybir as mb

    nc = _build(mt)
    bass2jax.install_neuronx_cc_hook()

    part_name = nc.partition_id_tensor.name if nc.partition_id_tensor else None
    in_names, out_names, out_avals = [], [], []
    for alloc in nc.m.functions[0].allocations:
        if not isinstance(alloc, mb.MemoryLocationSet):
            continue
        name = alloc.memorylocations[0].name
        if alloc.kind == "ExternalInput":
            if name != part_name:
                in_names.append(name)
        elif alloc.kind == "ExternalOutput":
            out_names.append(name)
            out_avals.append(
                jax.core.ShapedArray(tuple(alloc.tensor_shape),
                                     mb.dt.np(alloc.dtype)))
    n_params = len(in_names)
    n_outs = len(out_names)
    bind_names = in_names + out_names + ([part_name] if part_name else [])

    def _body(*args):
        operands = list(args)
        if part_name:
            operands.append(bass2jax.partition_id_tensor())
        outs = bass2jax._bass_exec_p.bind(
            *operands,
            out_avals=tuple(out_avals),
            in_names=tuple(bind_names),
            out_names=tuple(out_names),
            lowering_input_output_aliases=(),
            sim_require_finite=True,
            sim_require_nnan=True,
            nc=nc,
        )
        return tuple(outs)

    devices = jax.devices()[:NCORES]
    mesh = Mesh(np.asarray(devices), ("core",))
    sharded = jax.jit(
        shard_map(_body, mesh=mesh,
                  in_specs=(PartitionSpec("core"),) * (n_params + n_outs),
                  out_specs=(PartitionSpec("core"),) * n_outs,
                  check_rep=False),
        donate_argnums=tuple(range(n_params, n_params + n_outs)),
        keep_unused=True,
    )

    def run(in_maps):
        concat_in = [
            np.concatenate([m[name] for m in in_maps], axis=0)
            for name in in_names
        ]
        zeros = [np.zeros((NCORES * a.shape[0], *a.shape[1:]), a.dtype)
                 for a in out_avals]
        outs = sharded(*concat_in, *zeros)
        return [
            {name: np.asarray(outs[i]).reshape(NCORES, *out_avals[i].shape)[c]
             for i, name in enumerate(out_names)}
            for c in range(NCORES)
        ]

    return run


def kernel(x, W, modality_mapping):
    x = np.ascontiguousarray(np.asarray(x, dtype=np.float32))
    W = np.asarray(W, dtype=np.float32)
    mm = np.asarray(modality_mapping)

    perm = None
    if np.any(np.diff(mm) < 0):  # insurance: tokens not pre-sorted
        perm = np.argsort(mm, kind="stable")
        x = x[perm]
        mm = mm[perm]

    T = x.shape[0]
    E = W.shape[0]
    counts = np.bincount(mm.astype(np.int64), minlength=E)
    spans, t_max, n_mis = _plan_auto(counts, mm, T)
    mt = t_max // P

    if mt not in _cache:
        _cache[mt] = _runner(mt)
    run = _cache[mt]

    in_maps = _make_in_maps(x, W, spans, t_max)
    results = run(in_maps)

    out = np.empty((T, QKV_OUT), dtype=np.float32)
    ec = np.empty(T, dtype=np.int64)
    for c, (e, off, ln) in enumerate(spans):
        out[off:off + ln] = results[c]["out"][:ln]
        ec[off:off + ln] = e
    if n_mis:
        # Recompute boundary tokens (expert != core majority) exactly.
        bad = np.nonzero(mm.astype(np.int64) != ec)[0]
        for e in np.unique(mm[bad]):
            rows = bad[mm[bad] == e]
            out[rows] = x[rows] @ W[int(e)].T
    if perm is not None:
        inv = np.empty_like(perm)
        inv[perm] = np.arange(T)
        out = out[inv]
    return out
